# revision 34
# baseline (speedup 1.0000x reference)
"""Trainium2 Bass kernel for fused QKV-projection + multi-head attention.

Problem: x[2,2048,1024] @ W_qkv[1024,3072] + b -> split q/k/v -> 16 heads of
dim 64 -> softmax(q k^T / 8) v -> [2,2048,1024].

Sharding (8 cores): data-parallel over batch (2) x tensor-parallel over head
groups (4 heads per core).  Each core computes a disjoint output slice
[2048, 256]; no collectives are needed.

Design notes:
- Matmul operands are fp16 (fp32 PSUM accumulation).  x is pre-transposed and
  pre-cast on the host.
- q and k are stored pair-packed [128, T].  Scores for the two heads of a
  pair are K=64 row-tiled matmuls (array strips 0-1 / 2-3) that run
  concurrently; scoresT [k, q] layout keeps the softmax denominator on the
  PE (ones-column in the zero-padded [128,128] V weights).  exp has no
  max-subtraction: scores are bounded for this problem's scale.
- The attention loop is kb-outer over PAIRS of 512-wide q chunks, so every
  kT / V weight load feeds two matmuls (halves weight-switch overhead).
  AV lags one kb so exp never stalls the in-order PE queue.
- exp: 4 ops of [128,512] per kb, split ACT (true Exp) / DVE (Schraudolph
  bit-trick: u16 = 184.664*s + 15316 bitcast to fp16 ~= exp(s/8), ~2% rms,
  rounding-offset cancels in softmax) by (kb+s) parity -> both engines run
  at ~45%, and the overall rel err stays ~9e-3 (<2e-2).
- PSUM: 4 single-bank score slots (per chunk x head, reused kb->kb+1 with
  per-tile deps) + 4 AV accumulator banks; projections borrow the AV tags
  and run dense between attention passes (the PE is saturated either way).
- The last AV and the evacuation copies of each chunk-pair are deferred
  into the next super-iteration's first kbs so they never stall the PE.
- The kernel returns yT [256, T] (head-major, unnormalized) and den [4, T];
  the host divides and transposes.
"""

import sys

sys.path.insert(0, "/opt/trn_rl_repo")

import numpy as np

import concourse.bacc as bacc
import concourse.bass as bass
import concourse.mybir as mybir
import concourse.tile as tile
from concourse.bass import ts

P = 128
T = 2048
D = 1024
NH = 4          # heads per core
HD = 64         # head dim
TB = T // P     # 16 t-blocks
CB = D // P     # 8 c-blocks
QKV_COLS = 3 * NH * HD  # 768 per core
F32 = mybir.dt.float32
F16 = mybir.dt.float16
I16 = mybir.dt.int16

# Schraudolph exp(s/8) in fp16 bits: u16 = round(1024*log2(e)) + 15360 + c
SCH_MUL = 1024.0 * 0.125 * 1.4426950408889634   # 184.664
SCH_ADD = 15360.0 - 44.5 + 0.5                  # +0.5: trunc-to-floor comp

_CACHED = {}


def build_bass(finalize=True):
    nc = bacc.Bacc()

    xT_d = nc.dram_tensor("xT", [D, T], F16, kind="ExternalInput")
    w_d = nc.dram_tensor("w", [D, QKV_COLS], F16, kind="ExternalInput")
    bqk_d = nc.dram_tensor("bqk", [P, 4], F32, kind="ExternalInput")
    bv_d = nc.dram_tensor("bv", [1, NH * HD], F32, kind="ExternalInput")
    y_d = nc.dram_tensor("y", [2 * P, T], F32, kind="ExternalOutput")
    den_d = nc.dram_tensor("den", [NH, T], F32, kind="ExternalOutput")

    with tile.TileContext(nc) as tc:
        with (
            tc.tile_pool(name="persist", bufs=1) as persist,
            tc.tile_pool(name="small", bufs=2) as small,
            tc.tile_pool(name="ystage", bufs=4) as ystage,
            tc.tile_pool(name="epool", bufs=4) as epool,
            tc.tile_pool(name="ps", bufs=1, space="PSUM") as ps,
        ):
            # kT/qT: [p, t] pair-packed; head 2*pr at partitions 0:64,
            # head 2*pr+1 at 64:128
            kT = [persist.tile([P, T], F16, name=f"kT{i}") for i in range(2)]
            qT = [persist.tile([P, T], F16, name=f"qT{i}") for i in range(2)]
            # V' with ones column per head, zero-padded to 128 weight columns
            # so the AV lhsT is a full [128,128] load (the matmul runs at
            # M=128 for the same N cycles; rows 65:128 of the output are
            # garbage and never read): [t-part, h, 128], one per tb
            vv = [
                persist.tile([P, NH, P], F16, name=f"vv{tb}")
                for tb in range(TB)
            ]
            for tb in range(TB):
                nc.vector.memset(vv[tb][:, :, HD:], 0.0)
                nc.vector.memset(vv[tb][:, :, HD : HD + 1], 1.0)
            bqk_sb = persist.tile([P, 4], F32)
            bvb = persist.tile([P, NH * HD], F32)

            nc.sync.dma_start(out=bqk_sb[:], in_=bqk_d[:, :])
            nc.gpsimd.dma_start(
                out=bvb[:], in_=bv_d[0:1, :].to_broadcast((P, NH * HD))
            )

            # W split per column group so the first projections' weights land
            # before the whole W transfer completes
            wct = [
                persist.tile([P, CB, P], F16, name=f"wct{i}") for i in range(4)
            ]
            wv = persist.tile([P, CB, NH * HD], F16)
            # x lands t-first-half for all c-blocks first, so the first
            # projection chunks (and v_proj block 0) can start before the
            # whole x transfer completes; [64, 1024] pieces keep 2KB lines
            xTs = [persist.tile([P, T], F16, name=f"xTs{cb}") for cb in range(CB)]

            def dma_x(th):
                for cb in range(CB):
                    for hh in range(2):
                        nc.sync.dma_start(
                            out=xTs[cb][ts(hh, 64), ts(th, 1024)],
                            in_=xT_d[
                                cb * P + hh * 64 : cb * P + (hh + 1) * 64,
                                ts(th, 1024),
                            ],
                        )

            def dma_w(i):
                nc.sync.dma_start(
                    out=wct[i][:],
                    in_=w_d[:, ts(i, P)].rearrange("(cb p) col -> p cb col", p=P),
                )

            dma_w(2)
            dma_x(0)
            dma_w(0)
            warm = persist.tile([P, 512], F16, name="warm")
            nc.vector.memset(warm[:], 0.0)

            def warm_mm(n=1):
                # Y2 is free while qk_proj's first chunk-pair (Y0/Y1) runs
                for _ in range(n):
                    pw = ps.tile([P, 512], F32, tag="Y2", name="pw")
                    nc.tensor.matmul(
                        pw[:], lhsT=warm[:, 0:P], rhs=warm[:],
                        start=True, stop=True,
                    )
            nc.sync.dma_start(
                out=wv[:],
                in_=w_d[:, 2 * NH * HD :].rearrange("(cb p) col -> p cb col", p=P),
            )
            dma_x(1)
            dma_w(1)
            dma_w(3)

            # ---------------- QKV projection --------------------------------
            # ct: 0/1 = q pair 0/1, 2/3 = k pair 0/1.  Two 512-wide t-chunks
            # per weight load (accumulating into two psum banks) so each wct
            # LDWEIGHTS covers 2 matmuls.  Projections run dense (the PE is
            # the binding engine either way); psum borrows the Y tags, which
            # attention only uses later.
            def qk_proj(ct, warm_fill=False):
                dst = qT[ct] if ct < 2 else kT[ct - 2]
                for pi, tc2 in enumerate((0, 2)):
                    tags = ("Y0", "Y1") if pi % 2 == 0 else ("Y2", "Y3")
                    pqk = [
                        ps.tile([P, 512], F32, tag=tags[h], name="pqk")
                        for h in range(2)
                    ]
                    for cb in range(CB):
                        if warm_fill and pi == 0:
                            warm_mm()
                        for h in range(2):
                            nc.tensor.matmul(
                                pqk[h][:],
                                lhsT=wct[ct][:, cb, :],
                                rhs=xTs[cb][:, ts(tc2 + h, 512)],
                                start=(cb == 0),
                                stop=(cb == CB - 1),
                            )
                    for h in range(2):
                        if h == 0:
                            nc.vector.tensor_scalar_add(
                                out=dst[:, ts(tc2 + h, 512)],
                                in0=pqk[h][:],
                                scalar1=bqk_sb[:, ct : ct + 1],
                            )
                        else:
                            nc.scalar.add(
                                out=dst[:, ts(tc2 + h, 512)],
                                in_=pqk[h][:],
                                add=bqk_sb[:, ct : ct + 1],
                            )

            def v_proj(tb):
                ptag = ("Y0", "Y1", "Y2", "Y3")[tb % 4]
                pv = ps.tile([P, NH * HD], F32, tag=ptag, name="pv")
                for cb in range(CB):
                    nc.tensor.matmul(
                        pv[:],
                        lhsT=xTs[cb][:, ts(tb, P)],
                        rhs=wv[:, cb, :],
                        start=(cb == 0),
                        stop=(cb == CB - 1),
                    )
                nc.vector.tensor_tensor(
                    out=vv[tb][:, :, 0:HD],
                    in0=pv[:].rearrange("p (a b) -> p a b", a=NH),
                    in1=bvb[:].rearrange("p (a b) -> p a b", a=NH),
                    op=mybir.AluOpType.add,
                )

            # ---------------- attention -------------------------------------
            # kb-outer over PAIRS of 512-wide q chunks (A, B): each weight
            # load covers both chunks.  Per kb:
            #  - scores: two K=64 row-tiled matmuls per chunk (heads run
            #    CONCURRENTLY on row strips 0-1 / 2-3), kT loaded once/head.
            #  - exp: 4 ops of [128,512], ACT (true exp) / DVE (Schraudolph)
            #    split by (kb+s) parity.
            #  - AV(kb-1): 2-way COL-tiled (h0 -> output partitions 0:64,
            #    h1 -> 64:128, concurrent with separate e streams); both
            #    chunks reuse the loaded v weights.  pY = 1 bank per chunk.
            #  - den(kb-1): 4-way col-tiled M=1 ones-matmuls (all four
            #    (chunk, head) denominators concurrently, one bank, at
            #    partitions 0/32/64/96).
            # The 4 score slots are single-bank, reused kb -> kb+1 with
            # per-tile deps.  The last AV/den + evacuations are deferred into
            # the next super-iteration's first kbs so they never stall.
            pending = []

            def attention2(pr, qcp):
                qb = qcp * 2  # first 512-chunk index of this pair
                pY = {
                    (c, s): ps.tile(
                        [P, 512], F32, tag=f"Y{2 * c + s}", name=f"pY{c}{s}"
                    )
                    for c in range(2)
                    for s in range(2)
                }

                def issue_av(kb, epair, pY=pY, pr=pr):
                    for s in range(2):
                        for c in range(2):
                            nc.tensor.matmul(
                                pY[(c, s)][:],
                                lhsT=vv[kb][:, 2 * pr + s, :],
                                rhs=epair[c][:, ts(s, 512)],
                                start=(kb == 0),
                                stop=(kb == TB - 1),
                            )

                def make_evac(c, s, pY=pY, pr=pr, qb=qb):
                    def ev():
                        yst = ystage.tile([HD + 1, 512], F32, name="yst")
                        if (c + s) % 2 == 0:
                            nc.scalar.copy(
                                out=yst[:], in_=pY[(c, s)][0 : HD + 1, :]
                            )
                        else:
                            nc.vector.tensor_copy(
                                out=yst[:], in_=pY[(c, s)][0 : HD + 1, :]
                            )
                        nc.sync.dma_start(
                            out=y_d[
                                pr * P + s * HD : pr * P + (s + 1) * HD,
                                ts(qb + c, 512),
                            ],
                            in_=yst[0:HD, :],
                        )
                        nc.sync.dma_start(
                            out=den_d[2 * pr + s : 2 * pr + s + 1, ts(qb + c, 512)],
                            in_=yst[HD : HD + 1, :],
                        )

                    return ev

                e_hist = []
                for kb in range(TB):
                    pS = {
                        (c, s): ps.tile(
                            [P, 512], F32, tag=f"S{c}{s}", name=f"pS{c}{s}"
                        )
                        for c in range(2)
                        for s in range(2)
                    }
                    for s in range(2):  # one kT load per head, 2 chunks each
                        for c in range(2):
                            nc.tensor.matmul(
                                pS[(c, s)][:],
                                lhsT=kT[pr][ts(s, 64), ts(kb, P)],
                                rhs=qT[pr][ts(s, 64), ts(qb + c, 512)],
                                start=True,
                                stop=True,
                            )
                    epair = [
                        epool.tile([P, 2 * 512], F16, name=f"eT{c}")
                        for c in range(2)
                    ]
                    for c in range(2):
                        for s in range(2):
                            if (kb + s) % 2 == 0:
                                nc.scalar.activation(
                                    out=epair[c][:, ts(s, 512)],
                                    in_=pS[(c, s)][:],
                                    func=mybir.ActivationFunctionType.Exp,
                                    scale=0.125,
                                )
                            else:
                                nc.vector.tensor_scalar(
                                    out=epair[c][:, ts(s, 512)].bitcast(I16),
                                    in0=pS[(c, s)][:],
                                    scalar1=SCH_MUL,
                                    scalar2=SCH_ADD,
                                    op0=mybir.AluOpType.mult,
                                    op1=mybir.AluOpType.add,
                                )
                    e_hist.append(epair)
                    # deferred work from the previous chunk-pair: the final
                    # AV+den at kb=0, all evacuations by kb=1 -- everything
                    # must be issued before AV(0) rewrites the Y banks
                    if pending:
                        if kb == 0:
                            for _ in range(3):
                                if pending:
                                    pending.pop(0)()
                        elif kb == 1:
                            while pending:
                                pending.pop(0)()
                    if kb >= 1:
                        issue_av(kb - 1, e_hist[kb - 1])
                pending.append(
                    lambda eh=e_hist, ia=issue_av: ia(TB - 1, eh[TB - 1])
                )
                for c in range(2):
                    for s in range(2):
                        pending.append(make_evac(c, s))

            # kT pair-0 first (scores need all of k before any q chunk), then
            # q pair-0 and all of v; pair-1 projections between the two
            # attention passes
            warm_mm(18)
            qk_proj(2, warm_fill=True)
            qk_proj(0)
            for tb in range(TB):
                v_proj(tb)
            attention2(0, 0)
            attention2(0, 1)
            while pending:  # flush before proj reuses the Y banks
                pending.pop(0)()
            qk_proj(1)
            qk_proj(3)
            attention2(1, 0)
            attention2(1, 1)
            while pending:
                pending.pop(0)()

    if finalize:
        nc.finalize()
    return nc


def _shard_inputs(x, W_qkv, b_qkv):
    """Build per-core input maps. Core c: batch c//4, head group c%4."""
    x = np.asarray(x, dtype=np.float32)
    W = np.asarray(W_qkv, dtype=np.float32)
    b = np.asarray(b_qkv, dtype=np.float32)
    bf = np.float16
    xT = [np.ascontiguousarray(x[bi].T.astype(bf)) for bi in range(2)]
    in_maps = []
    for c in range(8):
        bi, hg = c // 4, c % 4
        cs = hg * 256  # column start within each of q/k/v blocks
        w_core = np.concatenate(
            [
                W[:, cs : cs + 256],
                W[:, D + cs : D + cs + 256],
                W[:, 2 * D + cs : 2 * D + cs + 256],
            ],
            axis=1,
        ).astype(bf)
        bqk = np.concatenate([b[cs : cs + 256], b[D + cs : D + cs + 256]])
        bqk = np.ascontiguousarray(bqk.reshape(4, 128).T)
        bv = np.ascontiguousarray(b[2 * D + cs : 2 * D + cs + 256].reshape(1, 256))
        in_maps.append(
            {
                "xT": xT[bi],
                "w": np.ascontiguousarray(w_core),
                "bqk": bqk,
                "bv": bv,
            }
        )
    return in_maps


def kernel(x, W_qkv, b_qkv, trace=False):
    from concourse.bass_utils import run_bass_kernel_spmd

    if "nc" not in _CACHED:
        _CACHED["nc"] = build_bass()
    nc = _CACHED["nc"]

    in_maps = _shard_inputs(x, W_qkv, b_qkv)
    res = run_bass_kernel_spmd(nc, in_maps, list(range(8)), trace=trace)
    _CACHED["last_result"] = res

    out = np.empty((2, T, D), dtype=np.float32)
    for c in range(8):
        bi, hg = c // 4, c % 4
        yT = res.results[c]["y"]  # [256, T] unnormalized, head-major
        den = res.results[c]["den"]  # [4, T]
        y = (yT.reshape(NH, HD, T) / den[:, None, :]).transpose(2, 0, 1)
        out[bi, :, hg * 256 : (hg + 1) * 256] = y.reshape(T, NH * HD)
    return out


if __name__ == "__main__":
    nc = build_bass()
    print("built ok")


# revision 35
# speedup vs baseline: 1.1631x; 1.1631x over previous
"""Trainium2 Bass kernel for fused QKV-projection + multi-head attention.

Problem: x[2,2048,1024] @ W_qkv[1024,3072] + b -> split q/k/v -> 16 heads of
dim 64 -> softmax(q k^T / 8) v -> [2,2048,1024].

Sharding (8 cores): data-parallel over batch (2) x tensor-parallel over head
groups (4 heads per core).  Each core computes a disjoint output slice
[2048, 256]; no collectives are needed.

Design notes:
- Matmul operands are fp16 (fp32 PSUM accumulation).  x is pre-transposed and
  pre-cast on the host.
- q and k are stored pair-packed [128, T].  Scores for the two heads of a
  pair are K=64 row-tiled matmuls (array strips 0-1 / 2-3) that run
  concurrently; scoresT [k, q] layout keeps the softmax denominator on the
  PE (ones-column in the zero-padded [128,128] V weights).  exp has no
  max-subtraction: scores are bounded for this problem's scale.
- The attention loop is kb-outer over PAIRS of 512-wide q chunks, so every
  kT / V weight load feeds two matmuls (halves weight-switch overhead).
  AV lags one kb so exp never stalls the in-order PE queue.
- exp: 4 ops of [128,512] per kb, split ACT (true Exp) / DVE (Schraudolph
  bit-trick: u16 = 184.664*s + 15316 bitcast to fp16 ~= exp(s/8), ~2% rms,
  rounding-offset cancels in softmax) by (kb+s) parity -> both engines run
  at ~45%, and the overall rel err stays ~9e-3 (<2e-2).
- PSUM: 4 single-bank score slots (per chunk x head, reused kb->kb+1 with
  per-tile deps) + 4 AV accumulator banks; projections borrow the AV tags
  and run dense between attention passes (the PE is saturated either way).
- The last AV and the evacuation copies of each chunk-pair are deferred
  into the next super-iteration's first kbs so they never stall the PE.
- The kernel returns yT [256, T] (head-major, unnormalized) and den [4, T];
  the host divides and transposes.
"""

import sys

sys.path.insert(0, "/opt/trn_rl_repo")

import numpy as np

import concourse.bacc as bacc
import concourse.bass as bass
import concourse.mybir as mybir
import concourse.tile as tile
from concourse.bass import ts

P = 128
T = 2048
D = 1024
NH = 4          # heads per core
HD = 64         # head dim
TB = T // P     # 16 t-blocks
CB = D // P     # 8 c-blocks
QKV_COLS = 3 * NH * HD  # 768 per core
F32 = mybir.dt.float32
F16 = mybir.dt.float16
I16 = mybir.dt.int16

# Schraudolph exp(s/8) in fp16 bits: u16 = round(1024*log2(e)) + 15360 + c
SCH_MUL = 1024.0 * 0.125 * 1.4426950408889634   # 184.664
SCH_ADD = 15360.0 - 44.5 + 0.5                  # +0.5: trunc-to-floor comp

_CACHED = {}


def build_bass(finalize=True):
    nc = bacc.Bacc()

    xT_d = nc.dram_tensor("xT", [D, T], F16, kind="ExternalInput")
    w_d = nc.dram_tensor("w", [D, QKV_COLS], F16, kind="ExternalInput")
    bqk_d = nc.dram_tensor("bqk", [P, 4], F32, kind="ExternalInput")
    bv_d = nc.dram_tensor("bv", [1, NH * HD], F32, kind="ExternalInput")
    y_d = nc.dram_tensor("y", [2 * P, T], F32, kind="ExternalOutput")
    den_d = nc.dram_tensor("den", [NH, T], F32, kind="ExternalOutput")

    with tile.TileContext(nc) as tc:
        with (
            tc.tile_pool(name="persist", bufs=1) as persist,
            tc.tile_pool(name="small", bufs=2) as small,
            tc.tile_pool(name="ystage", bufs=4) as ystage,
            tc.tile_pool(name="epool", bufs=4) as epool,
            tc.tile_pool(name="ps", bufs=1, space="PSUM") as ps,
        ):
            # kT/qT: [p, t] pair-packed; head 2*pr at partitions 0:64,
            # head 2*pr+1 at 64:128
            kT = [persist.tile([P, T], F16, name=f"kT{i}") for i in range(2)]
            qT = [persist.tile([P, T], F16, name=f"qT{i}") for i in range(2)]
            # V' with ones column per head, zero-padded to 128 weight columns
            # so the AV lhsT is a full [128,128] load (the matmul runs at
            # M=128 for the same N cycles; rows 65:128 of the output are
            # garbage and never read): [t-part, h, 128], one per tb
            vv = [
                persist.tile([P, NH, P], F16, name=f"vv{tb}")
                for tb in range(TB)
            ]
            for tb in range(TB):
                nc.vector.memset(vv[tb][:, :, HD:], 0.0)
                nc.vector.memset(vv[tb][:, :, HD : HD + 1], 1.0)
            bqk_sb = persist.tile([P, 4], F32)
            bvb = persist.tile([P, NH * HD], F32)

            nc.sync.dma_start(out=bqk_sb[:], in_=bqk_d[:, :])
            nc.gpsimd.dma_start(
                out=bvb[:], in_=bv_d[0:1, :].to_broadcast((P, NH * HD))
            )

            # W split per column group so the first projections' weights land
            # before the whole W transfer completes
            wct = [
                persist.tile([P, CB, P], F16, name=f"wct{i}") for i in range(4)
            ]
            wv = persist.tile([P, CB, NH * HD], F16)
            # x lands t-first-half for all c-blocks first, so the first
            # projection chunks (and v_proj block 0) can start before the
            # whole x transfer completes; [64, 1024] pieces keep 2KB lines
            xTs = [persist.tile([P, T], F16, name=f"xTs{cb}") for cb in range(CB)]

            def dma_x(th):
                # th kept for call-compat; each call moves half the c-blocks
                # as full [64, 2048] rows (4KB contiguous DMA lines)
                for cb in range(4 * th, 4 * th + 4):
                    for hh in range(2):
                        nc.sync.dma_start(
                            out=xTs[cb][ts(hh, 64), :],
                            in_=xT_d[cb * P + hh * 64 : cb * P + (hh + 1) * 64, :],
                        )

            def dma_w(i):
                nc.sync.dma_start(
                    out=wct[i][:],
                    in_=w_d[:, ts(i, P)].rearrange("(cb p) col -> p cb col", p=P),
                )

            dma_w(2)
            dma_x(0)
            dma_w(0)
            nc.sync.dma_start(
                out=wv[:],
                in_=w_d[:, 2 * NH * HD :].rearrange("(cb p) col -> p cb col", p=P),
            )
            dma_x(1)
            dma_w(1)
            dma_w(3)

            # ---------------- QKV projection --------------------------------
            # ct: 0/1 = q pair 0/1, 2/3 = k pair 0/1.  Two 512-wide t-chunks
            # per weight load (accumulating into two psum banks) so each wct
            # LDWEIGHTS covers 2 matmuls.  Projections run dense (the PE is
            # the binding engine either way); psum borrows the Y tags, which
            # attention only uses later.
            def qk_proj(ct):
                dst = qT[ct] if ct < 2 else kT[ct - 2]
                for pi, tc2 in enumerate((0, 2)):
                    tags = ("Y0", "Y1") if pi % 2 == 0 else ("Y2", "Y3")
                    pqk = [
                        ps.tile([P, 512], F32, tag=tags[h], name="pqk")
                        for h in range(2)
                    ]
                    for cb in range(CB):
                        for h in range(2):
                            nc.tensor.matmul(
                                pqk[h][:],
                                lhsT=wct[ct][:, cb, :],
                                rhs=xTs[cb][:, ts(tc2 + h, 512)],
                                start=(cb == 0),
                                stop=(cb == CB - 1),
                            )
                    for h in range(2):
                        if h == 0:
                            nc.vector.tensor_scalar_add(
                                out=dst[:, ts(tc2 + h, 512)],
                                in0=pqk[h][:],
                                scalar1=bqk_sb[:, ct : ct + 1],
                            )
                        else:
                            nc.scalar.add(
                                out=dst[:, ts(tc2 + h, 512)],
                                in_=pqk[h][:],
                                add=bqk_sb[:, ct : ct + 1],
                            )

            def v_proj(tb):
                ptag = ("Y0", "Y1", "Y2", "Y3")[tb % 4]
                pv = ps.tile([P, NH * HD], F32, tag=ptag, name="pv")
                for cb in range(CB):
                    nc.tensor.matmul(
                        pv[:],
                        lhsT=xTs[cb][:, ts(tb, P)],
                        rhs=wv[:, cb, :],
                        start=(cb == 0),
                        stop=(cb == CB - 1),
                    )
                nc.vector.tensor_tensor(
                    out=vv[tb][:, :, 0:HD],
                    in0=pv[:].rearrange("p (a b) -> p a b", a=NH),
                    in1=bvb[:].rearrange("p (a b) -> p a b", a=NH),
                    op=mybir.AluOpType.add,
                )

            # ---------------- attention -------------------------------------
            # kb-outer over PAIRS of 512-wide q chunks (A, B): each weight
            # load covers both chunks.  Per kb:
            #  - scores: two K=64 row-tiled matmuls per chunk (heads run
            #    CONCURRENTLY on row strips 0-1 / 2-3), kT loaded once/head.
            #  - exp: 4 ops of [128,512], ACT (true exp) / DVE (Schraudolph)
            #    split by (kb+s) parity.
            #  - AV(kb-1): 2-way COL-tiled (h0 -> output partitions 0:64,
            #    h1 -> 64:128, concurrent with separate e streams); both
            #    chunks reuse the loaded v weights.  pY = 1 bank per chunk.
            #  - den(kb-1): 4-way col-tiled M=1 ones-matmuls (all four
            #    (chunk, head) denominators concurrently, one bank, at
            #    partitions 0/32/64/96).
            # The 4 score slots are single-bank, reused kb -> kb+1 with
            # per-tile deps.  The last AV/den + evacuations are deferred into
            # the next super-iteration's first kbs so they never stall.
            pending = []

            def attention2(pr, qcp):
                qb = qcp * 2  # first 512-chunk index of this pair
                pY = {
                    (c, s): ps.tile(
                        [P, 512], F32, tag=f"Y{2 * c + s}", name=f"pY{c}{s}"
                    )
                    for c in range(2)
                    for s in range(2)
                }

                def issue_av(kb, epair, pY=pY, pr=pr):
                    for s in range(2):
                        for c in range(2):
                            nc.tensor.matmul(
                                pY[(c, s)][:],
                                lhsT=vv[kb][:, 2 * pr + s, :],
                                rhs=epair[c][:, ts(s, 512)],
                                start=(kb == 0),
                                stop=(kb == TB - 1),
                            )

                def make_evac(c, s, pY=pY, pr=pr, qb=qb):
                    def ev():
                        yst = ystage.tile([HD + 1, 512], F32, name="yst")
                        if (c + s) % 2 == 0:
                            nc.scalar.copy(
                                out=yst[:], in_=pY[(c, s)][0 : HD + 1, :]
                            )
                        else:
                            nc.vector.tensor_copy(
                                out=yst[:], in_=pY[(c, s)][0 : HD + 1, :]
                            )
                        nc.sync.dma_start(
                            out=y_d[
                                pr * P + s * HD : pr * P + (s + 1) * HD,
                                ts(qb + c, 512),
                            ],
                            in_=yst[0:HD, :],
                        )
                        nc.sync.dma_start(
                            out=den_d[2 * pr + s : 2 * pr + s + 1, ts(qb + c, 512)],
                            in_=yst[HD : HD + 1, :],
                        )

                    return ev

                e_hist = []
                for kb in range(TB):
                    pS = {
                        (c, s): ps.tile(
                            [P, 512], F32, tag=f"S{c}{s}", name=f"pS{c}{s}"
                        )
                        for c in range(2)
                        for s in range(2)
                    }
                    for s in range(2):  # one kT load per head, 2 chunks each
                        for c in range(2):
                            nc.tensor.matmul(
                                pS[(c, s)][:],
                                lhsT=kT[pr][ts(s, 64), ts(kb, P)],
                                rhs=qT[pr][ts(s, 64), ts(qb + c, 512)],
                                start=True,
                                stop=True,
                            )
                    epair = [
                        epool.tile([P, 2 * 512], F16, name=f"eT{c}")
                        for c in range(2)
                    ]
                    for c in range(2):
                        for s in range(2):
                            if (kb + s) % 2 == 0:
                                nc.scalar.activation(
                                    out=epair[c][:, ts(s, 512)],
                                    in_=pS[(c, s)][:],
                                    func=mybir.ActivationFunctionType.Exp,
                                    scale=0.125,
                                )
                            else:
                                nc.vector.tensor_scalar(
                                    out=epair[c][:, ts(s, 512)].bitcast(I16),
                                    in0=pS[(c, s)][:],
                                    scalar1=SCH_MUL,
                                    scalar2=SCH_ADD,
                                    op0=mybir.AluOpType.mult,
                                    op1=mybir.AluOpType.add,
                                )
                    e_hist.append(epair)
                    # deferred work from the previous chunk-pair: the final
                    # AV+den at kb=0, all evacuations by kb=1 -- everything
                    # must be issued before AV(0) rewrites the Y banks
                    if pending:
                        if kb == 0:
                            for _ in range(3):
                                if pending:
                                    pending.pop(0)()
                        elif kb == 1:
                            while pending:
                                pending.pop(0)()
                    if kb >= 1:
                        issue_av(kb - 1, e_hist[kb - 1])
                pending.append(
                    lambda eh=e_hist, ia=issue_av: ia(TB - 1, eh[TB - 1])
                )
                for c in range(2):
                    for s in range(2):
                        pending.append(make_evac(c, s))

            # kT pair-0 first (scores need all of k before any q chunk), then
            # q pair-0 and all of v; pair-1 projections between the two
            # attention passes
            qk_proj(2)
            qk_proj(0)
            for tb in range(TB):
                v_proj(tb)
            attention2(0, 0)
            attention2(0, 1)
            while pending:  # flush before proj reuses the Y banks
                pending.pop(0)()
            qk_proj(1)
            qk_proj(3)
            attention2(1, 0)
            attention2(1, 1)
            while pending:
                pending.pop(0)()

    if finalize:
        nc.finalize()
    return nc


def _shard_inputs(x, W_qkv, b_qkv):
    """Build per-core input maps. Core c: batch c//4, head group c%4."""
    x = np.asarray(x, dtype=np.float32)
    W = np.asarray(W_qkv, dtype=np.float32)
    b = np.asarray(b_qkv, dtype=np.float32)
    bf = np.float16
    xT = [np.ascontiguousarray(x[bi].T.astype(bf)) for bi in range(2)]
    in_maps = []
    for c in range(8):
        bi, hg = c // 4, c % 4
        cs = hg * 256  # column start within each of q/k/v blocks
        w_core = np.concatenate(
            [
                W[:, cs : cs + 256],
                W[:, D + cs : D + cs + 256],
                W[:, 2 * D + cs : 2 * D + cs + 256],
            ],
            axis=1,
        ).astype(bf)
        bqk = np.concatenate([b[cs : cs + 256], b[D + cs : D + cs + 256]])
        bqk = np.ascontiguousarray(bqk.reshape(4, 128).T)
        bv = np.ascontiguousarray(b[2 * D + cs : 2 * D + cs + 256].reshape(1, 256))
        in_maps.append(
            {
                "xT": xT[bi],
                "w": np.ascontiguousarray(w_core),
                "bqk": bqk,
                "bv": bv,
            }
        )
    return in_maps


def kernel(x, W_qkv, b_qkv, trace=False):
    from concourse.bass_utils import run_bass_kernel_spmd

    if "nc" not in _CACHED:
        _CACHED["nc"] = build_bass()
    nc = _CACHED["nc"]

    in_maps = _shard_inputs(x, W_qkv, b_qkv)
    res = run_bass_kernel_spmd(nc, in_maps, list(range(8)), trace=trace)
    _CACHED["last_result"] = res

    out = np.empty((2, T, D), dtype=np.float32)
    for c in range(8):
        bi, hg = c // 4, c % 4
        yT = res.results[c]["y"]  # [256, T] unnormalized, head-major
        den = res.results[c]["den"]  # [4, T]
        y = (yT.reshape(NH, HD, T) / den[:, None, :]).transpose(2, 0, 1)
        out[bi, :, hg * 256 : (hg + 1) * 256] = y.reshape(T, NH * HD)
    return out


if __name__ == "__main__":
    nc = build_bass()
    print("built ok")
